# revision 10
# baseline (speedup 1.0000x reference)
"""Distributed euclidean-distance loss kernel for Trainium2 (8 NeuronCores).

loss = sum_i sqrt(sum_c (preds[i,c] - targets[i,c])^2) / (N + 1)

preds/targets: [16777216, 2] f32. Data-parallel over the batch axis:
each of the 8 cores reduces its 1/8 slice to a per-partition partial
sum [128, 1]; the host sums the 8x128 partials and divides by N+1.

Host-side sharding packs preds and targets tiles into one interleaved
DRAM tensor per core ([... ntiles, 2, f]) so each SBUF tile is filled
by a single large DMA.
"""

import numpy as np

import concourse.bass as bass
import concourse.bacc as bacc
import concourse.mybir as mybir
import concourse.tile as tile
from concourse import dve_ops
from concourse.bass_utils import run_bass_kernel_spmd
from concourse.dve_spec import Spec, Src0, Src1, _has_src1, lower, sq
from concourse.dve_uop import DveOpSpec

N_CORES = 8
N_POINTS = 16777216          # total points
PTS_PER_CORE = N_POINTS // N_CORES          # 2_097_152
ELEMS_PER_CORE = PTS_PER_CORE * 2           # 4_194_304 floats per tensor
P = 128                                      # SBUF partitions
M = ELEMS_PER_CORE // P                      # 32768 floats per partition
F = 2048                                     # tile free size per tensor
NTILES = M // F                              # 16

_cache = {}


def _register_sqdiff():
    """Register a custom DVE op out = (in0 - in1)^2 so the subtract+square
    is one Vector instruction (no ScalarE hop inside the per-tile chain)."""
    name = "SQDIFF_DIST_ANT"
    for op in dve_ops.OPS:
        if op.name == name:
            return op
    spec = Spec(
        body=sq(Src0 - Src1),
        reference=lambda in0, in1, s0, s1, imm2: (
            (in0.astype(np.float32) - in1) ** 2
        ).astype(np.float32),
    )
    row = max(dve_ops._SUB_OPCODE_FOR_NAME.values()) + 1
    assert row < 0x20
    shas = {}
    for ver in ("v3", "v4"):
        uops = lower(spec, ver=ver)
        shas[ver] = DveOpSpec(
            name=name, opcode=row, uops=uops, rd1_en=_has_src1(spec)
        ).sha(ver)
    op = dve_ops.DveOp(name, spec, subdim=False, uops_sha=shas)
    dve_ops.OPS.append(op)
    dve_ops._SUB_OPCODE_FOR_NAME[name] = row
    dve_ops.CUSTOM_DVE_SPECS[name] = spec
    return op


_SQDIFF = _register_sqdiff()


def _build(m=M, f=F):
    """Build the per-core Bass program. m = floats per partition per tensor."""
    ntiles = m // f
    fp32 = mybir.dt.float32
    nc = bacc.Bacc(
        "TRN2", target_bir_lowering=False, debug=False, num_devices=N_CORES
    )
    x_in = nc.declare_dram_parameter("x", [P, 2 * m], fp32, isOutput=False)
    out = nc.declare_dram_parameter("o", [P, 1], fp32, isOutput=True)

    with tile.TileContext(nc) as tc:
        with (
            tc.tile_pool(name="inp", bufs=5) as inp,
            tc.tile_pool(name="work", bufs=4) as work,
            tc.tile_pool(name="res", bufs=1) as res,
        ):
            acc = res.tile([P, ntiles], fp32, tag="acc")
            for i in range(ntiles):
                xt = inp.tile([P, 2 * f], fp32, tag="x")
                nc.sync.dma_start(out=xt[:], in_=x_in[:, bass.ts(i, 2 * f)])

                sq = work.tile([P, f], fp32, tag="sq")
                nc.vector._custom_dve(
                    _SQDIFF, out=sq[:], in0=xt[:, :f], in1=xt[:, f:]
                )
                ps = work.tile([P, f // 2], fp32, tag="ps")
                nc.vector.tensor_add(ps[:], sq[:, 0::2], sq[:, 1::2])
                nc.scalar.activation(
                    ps[:], ps[:], mybir.ActivationFunctionType.Sqrt,
                    accum_out=acc[:, i : i + 1],
                )
            total = res.tile([P, 1], fp32, tag="total")
            nc.vector.reduce_sum(total[:], acc[:], axis=mybir.AxisListType.X)
            nc.sync.dma_start(out=out[:], in_=total[:])
    nc.compile()
    return nc


def _pack(preds, targets, m, f, n_cores):
    """[N,2]x2 f32 -> per-core interleaved [n_cores, P, 2m] (tile-granular)."""
    ntiles = m // f
    p4 = np.ascontiguousarray(preds, dtype=np.float32).reshape(n_cores, P, ntiles, f)
    t4 = np.ascontiguousarray(targets, dtype=np.float32).reshape(n_cores, P, ntiles, f)
    x = np.empty((n_cores, P, ntiles, 2, f), dtype=np.float32)
    x[:, :, :, 0, :] = p4
    x[:, :, :, 1, :] = t4
    return x.reshape(n_cores, P, 2 * m)


def _run(preds, targets, m=M, f=F, n_cores=N_CORES, **run_kwargs):
    """Shard, run on hardware, return (partials [n_cores,128,1], results)."""
    key = (m, f)
    if key not in _cache:
        _cache[key] = _build(m, f)
    nc = _cache[key]
    x = _pack(preds, targets, m, f, n_cores)
    in_maps = [{"x": x[c]} for c in range(n_cores)]
    r = run_bass_kernel_spmd(nc, in_maps, core_ids=list(range(n_cores)), **run_kwargs)
    partials = np.stack([r.results[c]["o"] for c in range(n_cores)])
    return partials, r


def kernel(preds, targets):
    partials, _ = _run(preds, targets)
    n = preds.shape[0]
    loss = partials.astype(np.float64).sum() / np.float64(n + 1)
    return np.float32(loss)
